# revision 5
# baseline (speedup 1.0000x reference)
"""Trainium2 Bass kernel for nn_BartCrossAttention (B=4, L=1024, D=1024, H=16, HD=64).

Sharding: 8 cores; core c handles query tokens [512c, 512c+512) of batch b=c//2.
Each core recomputes K/V projections for its whole batch (no collectives); the
host slices inputs per core and concatenates outputs.

Design notes:
- All matmul operands bf16 (PE still 1 cycle/row, halves DMA+SBUF traffic;
  measured end-to-end rel err ~6e-3 vs the 2e-2 budget).
- hid/kv transposed on the HOST - no on-device PE transposes at all.
- ctx matmuls are software-pipelined one t-iteration behind S/exp so the PE
  never waits on the ACT engine inside an iteration.
- Softmax normalization is off-PE: ones column in the ctx matmul gives the
  denominator row, reciprocal_approx_fast on DVE, gpsimd partition_broadcast,
  and the PSUM->SBUF ctx eviction does the multiply (normalize-on-evict).
- K/Q projections for pair hp+1 interleave into pair hp's t-loop; during the
  last pair (no projections left) the first two out-projection chunks
  pre-accumulate fj=0..6 so the PE stays fed while ACT drains.
- DMA dispatches are ordered first-needed-first (each dma_start costs ~1us of
  Sync dispatch); ACT exp table is pre-warmed in the prologue.
"""
import sys

for _p in ("/opt/trn_rl_repo",):
    if _p not in sys.path:
        sys.path.insert(0, _p)

import numpy as np
import ml_dtypes

import concourse.bass as bass
import concourse.mybir as mybir
import concourse.tile as tile
from concourse import bacc
import concourse.bass_utils as bass_utils

F32 = mybir.dt.float32
BF16 = mybir.dt.bfloat16

P = 128
D = 1024        # model dim
H = 16          # heads
NCORES = 8
TQ = 512        # query tokens per core
LK = 1024       # kv tokens per batch
B, LQ = 4, 1024

_CACHE = {}


def _build_core_program():
    nc = bacc.Bacc("TRN2", target_bir_lowering=False, debug=False,
                   num_devices=NCORES)

    hid_t = nc.dram_tensor("hid_t", [D, TQ], BF16, kind="ExternalInput")
    kv_t = nc.dram_tensor("kv_t", [D, LK], BF16, kind="ExternalInput")
    wq_t = nc.dram_tensor("wq_t", [D, D], BF16, kind="ExternalInput")
    wk_t = nc.dram_tensor("wk_t", [D, D], BF16, kind="ExternalInput")
    wv_t = nc.dram_tensor("wv_t", [D, D], BF16, kind="ExternalInput")
    wo_t = nc.dram_tensor("wo_t", [D, D], BF16, kind="ExternalInput")
    qb_d = nc.dram_tensor("qb", [D], F32, kind="ExternalInput")
    kb_d = nc.dram_tensor("kb", [D], F32, kind="ExternalInput")
    vb_d = nc.dram_tensor("vb", [D], F32, kind="ExternalInput")
    ob_d = nc.dram_tensor("ob", [D], F32, kind="ExternalInput")
    out_s = nc.dram_tensor("out_s", [TQ, D], F32, kind="ExternalOutput")

    Exp = mybir.ActivationFunctionType.Exp
    add = mybir.AluOpType.add
    mult = mybir.AluOpType.mult

    with tile.TileContext(nc) as tc:
        with (
            tc.tile_pool(name="setup", bufs=1) as setup,
            tc.tile_pool(name="big", bufs=1) as big,
            tc.tile_pool(name="attn", bufs=4) as attnp,
            tc.tile_pool(name="norm", bufs=2) as normp,
            tc.tile_pool(name="outp", bufs=2) as outp,
            tc.tile_pool(name="pssc", bufs=2, space="PSUM") as pssc,
            tc.tile_pool(name="psctx", bufs=4, space="PSUM") as psctx,
            tc.tile_pool(name="psmm", bufs=2, space="PSUM") as psmm,
        ):
            # ---- persistent big tiles ----
            kvT = big.tile([P, 8, LK], BF16, tag="kvT")      # kv^T [1024,1024]
            hidT = big.tile([P, 8, TQ], BF16, tag="hidT")    # hid^T [1024,512]
            wv = big.tile([P, 8, D], BF16, tag="wv")
            wk = big.tile([P, 8, D], BF16, tag="wk")
            wq = big.tile([P, 8, D], BF16, tag="wq")
            wo = big.tile([P, 8, D], BF16, tag="wo")
            KT = big.tile([P, 8, LK], BF16, tag="KT")        # K^T per pair
            qT = big.tile([P, 8, TQ], BF16, tag="qT")        # Q^T per pair
            v65 = big.tile([P, 8, H * 65], BF16, tag="v65")  # V+ones col
            ctxT = big.tile([P, 8, TQ], BF16, tag="ctxT")    # normalized ctx^T

            # ---- DMA dispatch order: first-needed first ----
            kv_re = kv_t.ap().rearrange("(dd p) t -> p dd t", p=P)
            wv_re = wv_t.ap().rearrange("(dd p) o -> p dd o", p=P)
            nc.sync.dma_start(kvT[:, :, 0:512], kv_re[:, :, 0:512])
            nc.sync.dma_start(wv[:, 0:4, 0:512], wv_re[:, 0:4, 0:512])
            nc.sync.dma_start(wv[:, 4:8, 0:512], wv_re[:, 4:8, 0:512])
            nc.sync.dma_start(kvT[:, :, 512:1024], kv_re[:, :, 512:1024])
            nc.sync.dma_start(wv[:, :, 512:1024], wv_re[:, :, 512:1024])
            nc.sync.dma_start(wk[:], wk_t.ap().rearrange("(dd p) o -> p dd o", p=P))
            nc.sync.dma_start(hidT[:], hid_t.ap().rearrange("(dd p) t -> p dd t", p=P))
            nc.sync.dma_start(wq[:], wq_t.ap().rearrange("(dd p) o -> p dd o", p=P))
            nc.sync.dma_start(wo[:], wo_t.ap().rearrange("(dd p) o -> p dd o", p=P))

            # ---- setup: biases, ones, ACT table warm ----
            qb_sb = setup.tile([P, 8], F32, tag="qb")
            nc.sync.dma_start(qb_sb[:], qb_d.ap().rearrange("(o p) -> p o", p=P))
            kb_sb = setup.tile([P, 8], F32, tag="kb")
            nc.sync.dma_start(kb_sb[:], kb_d.ap().rearrange("(o p) -> p o", p=P))
            vbB = setup.tile([P, D], F32, tag="vbB")
            obB = setup.tile([P, D], F32, tag="obB")
            vb_row = setup.tile([1, D], F32, tag="vbrow")
            nc.sync.dma_start(vb_row[:], vb_d.ap()[None, :])
            nc.gpsimd.partition_broadcast(vbB[:], vb_row[:])
            ob_row = setup.tile([1, D], F32, tag="obrow")
            nc.sync.dma_start(ob_row[:], ob_d.ap()[None, :])
            nc.gpsimd.partition_broadcast(obB[:], ob_row[:])

            onesF = setup.tile([P, P], F32, tag="onesF")
            nc.gpsimd.memset(onesF[:], 1.0)
            warm = setup.tile([1, 8], BF16, tag="warm")
            nc.scalar.activation(warm[:], onesF[0:1, 0:8], Exp)

            # ones columns of v65 (col 64 of each head block)
            nc.vector.tensor_copy(
                v65[:].rearrange("p t (h x) -> p t h x", x=65)[:, :, :, 64:65],
                onesF[:].rearrange("p (t h x) -> p t h x", t=8, h=16))

            # ---- V projection ----
            v65v = v65[:].rearrange("p t (h x) -> p t h x", x=65)
            for half in range(2):
                for ti in range(8):
                    pp = psmm.tile([P, 512], F32, tag="pp",
                                   name=f"ppv{half}_{ti}")
                    for di in range(8):
                        nc.tensor.matmul(
                            pp[:],
                            kvT[:, di, ti * P:(ti + 1) * P],
                            wv[:, di, half * 512:(half + 1) * 512],
                            start=(di == 0), stop=(di == 7),
                        )
                    nc.vector.tensor_tensor(
                        v65v[:, ti, half * 8:(half + 1) * 8, 0:64], pp[:],
                        vbB[:, half * 512:(half + 1) * 512], add)

            # ---- K/Q projections (pair 0 now, rest interleaved) ----
            def emit_kproj(hp, nk):
                pp = psmm.tile([P, 512], F32, tag="pp", name=f"ppk{hp}_{nk}")
                for di in range(8):
                    nc.tensor.matmul(
                        pp[:],
                        wk[:, di, hp * P:(hp + 1) * P],
                        kvT[:, di, nk * 512:(nk + 1) * 512],
                        start=(di == 0), stop=(di == 7),
                    )
                nc.vector.tensor_scalar(
                    KT[:, hp, nk * 512:(nk + 1) * 512], pp[:],
                    kb_sb[:, hp:hp + 1], None, add)

            def emit_qproj(hp):
                pq = psmm.tile([P, 512], F32, tag="pp", name=f"ppq{hp}")
                for di in range(8):
                    nc.tensor.matmul(
                        pq[:],
                        wq[:, di, hp * P:(hp + 1) * P],
                        hidT[:, di, :],
                        start=(di == 0), stop=(di == 7),
                    )
                nc.vector.tensor_scalar(qT[:, hp, :], pq[:],
                                        qb_sb[:, hp:hp + 1], None, add)

            emit_kproj(0, 0)
            emit_kproj(0, 1)
            emit_qproj(0)

            # normalization: all off-PE (DVE recip + gpsimd bcast + DVE mult)
            def emit_norm(hp, ctx_ps):
                for hh in range(2):
                    cp = ctx_ps[hh]
                    rrow = normp.tile([65, 512], F32, tag="rrow",
                                      name=f"rrow{hp}_{hh}")
                    nc.vector.reciprocal(rrow[64:65, :], cp[64:65, :])
                    r0 = normp.tile([1, 512], F32, tag="r0",
                                    name=f"r0_{hp}_{hh}")
                    nc.sync.dma_start(r0[:], rrow[64:65, :])
                    bc = normp.tile([64, 512], F32, tag="bc",
                                    name=f"bc{hp}_{hh}")
                    nc.gpsimd.partition_broadcast(bc[:], r0[:])
                    if hh == 0:
                        nc.vector.tensor_tensor(
                            ctxT[0:64, hp, :], cp[0:64, :], bc[:], mult)
                    else:
                        stg = normp.tile([64, 512], BF16, tag="stg",
                                         name=f"stg{hp}")
                        nc.vector.tensor_tensor(stg[:], cp[0:64, :], bc[:], mult)
                        nc.sync.dma_start(ctxT[64:128, hp, :], stg[:])

            # out-projection chunk helpers (epilogue + hp7 partials)
            def o_chunk_matmuls(po, half, mi, fjs, start0, stop7):
                for fj in fjs:
                    nc.tensor.matmul(
                        po[:],
                        ctxT[:, fj, mi * P:(mi + 1) * P],
                        wo[:, fj, half * 512:(half + 1) * 512],
                        start=(fj == 0 and start0), stop=(fj == 7 and stop7),
                    )

            def o_chunk_finish(po, half, mi):
                ot = outp.tile([P, 512], F32, tag="ot")
                nc.vector.tensor_tensor(
                    ot[:], po[:], obB[:, half * 512:(half + 1) * 512], add)
                nc.sync.dma_start(
                    out_s.ap().rearrange("(mm p) d -> p mm d", p=P)[
                        :, mi, half * 512:(half + 1) * 512],
                    ot[:])

            # ---- main attention loop (ctx pipelined 1 iter behind) ----
            ctx_tiles = {}
            pend = None  # (hp, t, [at_e, at_o])
            opart = []   # hp7 partial out-proj chunks: (po, half, mi)

            def emit_ctx(hp, t, ats):
                for hh in range(2):
                    h = 2 * hp + hh
                    nc.tensor.matmul(
                        ctx_tiles[hp][hh][:],
                        v65[:, t, h * 65:(h + 1) * 65],
                        ats[hh][:],
                        start=(t == 0), stop=(t == 7),
                    )

            for hp in range(8):
                nxt = hp + 1
                ctx_tiles[hp] = [psctx.tile([65, 512], F32, tag="ctx",
                                            name=f"ctx{hp}_{i}")
                                 for i in range(2)]
                for t in range(8):
                    ats = []
                    for hh in range(2):
                        lo = 64 * hh
                        sc = pssc.tile([P, 512], F32, tag="sc",
                                       name=f"sc{hp}_{t}_{hh}")
                        nc.tensor.matmul(
                            sc[:],
                            KT[lo:lo + 64, hp, t * P:(t + 1) * P],
                            qT[lo:lo + 64, hp, :],
                            start=True, stop=True,
                        )
                        at = attnp.tile([P, 512], BF16, tag="at")
                        nc.scalar.activation(at[:], sc[:], Exp)
                        ats.append(at)
                    if pend is not None:
                        phh, pt, pats = pend
                        emit_ctx(phh, pt, pats)
                        if pt == 7:
                            emit_norm(phh, ctx_tiles[phh])
                    if nxt < 8:
                        if t == 2:
                            emit_kproj(nxt, 0)
                        elif t == 4:
                            emit_kproj(nxt, 1)
                        elif t == 6:
                            emit_qproj(nxt)
                    else:
                        # keep PE fed while ACT drains: pre-accumulate the
                        # first two out-proj chunks over fj=0..6
                        if t == 2 or t == 4:
                            mi = 0 if t == 2 else 1
                            po = psmm.tile([P, 512], F32, tag="pp",
                                           name=f"ppo0_{mi}")
                            o_chunk_matmuls(po, 0, mi, range(7), True, False)
                            opart.append((po, 0, mi))
                    pend = (hp, t, ats)

            emit_ctx(7, 7, pend[2])
            emit_norm(7, ctx_tiles[7])

            # ---- epilogue: finish out projection ----
            for po, half, mi in opart:
                o_chunk_matmuls(po, half, mi, [7], False, True)
                o_chunk_finish(po, half, mi)
            for half in range(2):
                for mi in range(4):
                    if half == 0 and mi < 2:
                        continue
                    po = psmm.tile([P, 512], F32, tag="pp",
                                   name=f"ppo{half}_{mi}")
                    o_chunk_matmuls(po, half, mi, range(8), True, True)
                    o_chunk_finish(po, half, mi)

    nc.compile()
    return nc


def _prep_inputs(hidden_states, key_value_states, q_weight, q_bias,
                 kv_weight, kv_bias, out_weight, out_bias):
    f32 = np.float32
    bf16 = ml_dtypes.bfloat16
    hid = np.asarray(hidden_states, f32).reshape(B * LQ, D)
    kv = np.asarray(key_value_states, f32).reshape(B * LK, D)
    scale = f32(1.0 / 8.0)

    # de-interleave kv rows: row e <-> (h=e//128, j=(e%128)//64, d=e%64)
    e = np.arange(2 * D)
    kmask = (e % 128) < 64
    kidx, vidx = e[kmask], e[~kmask]
    kvw = np.asarray(kv_weight, f32)
    kvb = np.asarray(kv_bias, f32)

    shared = {
        "wq_t": np.ascontiguousarray((np.asarray(q_weight, f32) * scale).T.astype(bf16)),
        "wk_t": np.ascontiguousarray(kvw[kidx].T.astype(bf16)),
        "wv_t": np.ascontiguousarray(kvw[vidx].T.astype(bf16)),
        "wo_t": np.ascontiguousarray(np.asarray(out_weight, f32).T.astype(bf16)),
        "qb": np.ascontiguousarray(np.asarray(q_bias, f32) * scale),
        "kb": np.ascontiguousarray(kvb[kidx]),
        "vb": np.ascontiguousarray(kvb[vidx]),
        "ob": np.ascontiguousarray(np.asarray(out_bias, f32)),
    }
    kvT_by_batch = [
        np.ascontiguousarray(kv[b * LK:(b + 1) * LK].T.astype(bf16))
        for b in range(B)
    ]
    in_maps = []
    for c in range(NCORES):
        b = c // 2
        m = dict(shared)
        m["hid_t"] = np.ascontiguousarray(
            hid[c * TQ:(c + 1) * TQ].T.astype(bf16))
        m["kv_t"] = kvT_by_batch[b]
        in_maps.append(m)
    return in_maps


def kernel(hidden_states, key_value_states, q_weight, q_bias,
           kv_weight, kv_bias, out_weight, out_bias, _trace=False):
    if "nc" not in _CACHE:
        _CACHE["nc"] = _build_core_program()
    nc = _CACHE["nc"]
    in_maps = _prep_inputs(hidden_states, key_value_states, q_weight, q_bias,
                           kv_weight, kv_bias, out_weight, out_bias)
    res = bass_utils.run_bass_kernel_spmd(
        nc, in_maps, core_ids=list(range(NCORES)), trace=_trace)
    _CACHE["last_result"] = res
    out = np.concatenate([r["out_s"] for r in res.results], axis=0)
    return out.reshape(B, LQ, D)


# revision 8
# speedup vs baseline: 1.1488x; 1.1488x over previous
"""Trainium2 Bass kernel for nn_BartCrossAttention (B=4, L=1024, D=1024, H=16, HD=64).

Sharding: 8 cores; core c handles query tokens [512c, 512c+512) of batch b=c//2.
Each core recomputes K/V projections for its whole batch (no collectives); the
host slices inputs per core and concatenates outputs.

Design notes:
- All matmul operands bf16 (PE still 1 cycle/row, halves DMA+SBUF traffic;
  measured end-to-end rel err ~6e-3 vs the 2e-2 budget).
- hid/kv transposed on the HOST - no on-device PE transposes at all.
- ctx matmuls are software-pipelined one t-iteration behind S/exp so the PE
  never waits on the ACT engine inside an iteration.
- Softmax normalization is off-PE: ones column in the ctx matmul gives the
  denominator row, reciprocal_approx_fast on DVE, gpsimd partition_broadcast,
  and the PSUM->SBUF ctx eviction does the multiply (normalize-on-evict).
- K/Q projections for pair hp+1 interleave into pair hp's t-loop; during the
  last pair (no projections left) the first two out-projection chunks
  pre-accumulate fj=0..6 so the PE stays fed while ACT drains.
- DMA dispatches are ordered first-needed-first (each dma_start costs ~1us of
  Sync dispatch); ACT exp table is pre-warmed in the prologue.
"""
import sys

for _p in ("/opt/trn_rl_repo",):
    if _p not in sys.path:
        sys.path.insert(0, _p)

import numpy as np
import ml_dtypes

import concourse.bass as bass
import concourse.mybir as mybir
import concourse.tile as tile
from concourse import bacc
import concourse.bass_utils as bass_utils

F32 = mybir.dt.float32
BF16 = mybir.dt.bfloat16

P = 128
D = 1024        # model dim
H = 16          # heads
NCORES = 8
TQ = 512        # query tokens per core
LK = 1024       # kv tokens per batch
B, LQ = 4, 1024

_CACHE = {}


def _build_core_program():
    nc = bacc.Bacc("TRN2", target_bir_lowering=False, debug=False,
                   num_devices=NCORES)

    hid_t = nc.dram_tensor("hid_t", [D, TQ], BF16, kind="ExternalInput")
    kv_t = nc.dram_tensor("kv_t", [D, LK], BF16, kind="ExternalInput")
    wq_t = nc.dram_tensor("wq_t", [D, D], BF16, kind="ExternalInput")
    wk_t = nc.dram_tensor("wk_t", [D, D], BF16, kind="ExternalInput")
    wv_t = nc.dram_tensor("wv_t", [D, D], BF16, kind="ExternalInput")
    wo_t = nc.dram_tensor("wo_t", [D, D], BF16, kind="ExternalInput")
    qb_d = nc.dram_tensor("qb", [D], F32, kind="ExternalInput")
    kb_d = nc.dram_tensor("kb", [D], F32, kind="ExternalInput")
    vb_d = nc.dram_tensor("vb", [D], F32, kind="ExternalInput")
    ob_d = nc.dram_tensor("ob", [D], F32, kind="ExternalInput")
    out_s = nc.dram_tensor("out_s", [TQ, D], F32, kind="ExternalOutput")

    Exp = mybir.ActivationFunctionType.Exp
    add = mybir.AluOpType.add
    mult = mybir.AluOpType.mult

    with tile.TileContext(nc) as tc:
        with (
            tc.tile_pool(name="setup", bufs=1) as setup,
            tc.tile_pool(name="big", bufs=1) as big,
            tc.tile_pool(name="attn", bufs=4) as attnp,
            tc.tile_pool(name="norm", bufs=2) as normp,
            tc.tile_pool(name="outp", bufs=2) as outp,
            tc.tile_pool(name="pssc", bufs=2, space="PSUM") as pssc,
            tc.tile_pool(name="psctx", bufs=4, space="PSUM") as psctx,
            tc.tile_pool(name="psmm", bufs=2, space="PSUM") as psmm,
        ):
            # ---- persistent big tiles ----
            kvT = big.tile([P, 8, LK], BF16, tag="kvT")      # kv^T [1024,1024]
            hidT = big.tile([P, 8, TQ], BF16, tag="hidT")    # hid^T [1024,512]
            wv = big.tile([P, 8, D], BF16, tag="wv")
            wk = big.tile([P, 8, D], BF16, tag="wk")
            wq = big.tile([P, 8, D], BF16, tag="wq")
            wo = big.tile([P, 8, D], BF16, tag="wo")
            KT = big.tile([P, 8, LK], BF16, tag="KT")        # K^T per pair
            qT = big.tile([P, 8, TQ], BF16, tag="qT")        # Q^T per pair
            v65 = big.tile([P, 8, H * 65], BF16, tag="v65")  # V+ones col
            ctxT = big.tile([P, 8, TQ], BF16, tag="ctxT")    # normalized ctx^T

            # ---- DMA dispatch order: first-needed first. The DMA engine
            # round-robins all pending transfers, so later bulk loads are
            # token-gated (1-elem DVE copy into the dst tile = WAR dep) to
            # keep them from stealing bandwidth from the critical prologue.
            kv_re = kv_t.ap().rearrange("(dd p) t -> p dd t", p=P)
            wv_re = wv_t.ap().rearrange("(dd p) o -> p dd o", p=P)
            nc.sync.dma_start(kvT[:, :, 0:512], kv_re[:, :, 0:512])
            nc.sync.dma_start(wv[:, :, 0:512], wv_re[:, :, 0:512])
            nc.sync.dma_start(kvT[:, :, 512:1024], kv_re[:, :, 512:1024])
            nc.sync.dma_start(wv[:, :, 512:1024], wv_re[:, :, 512:1024])

            # ---- setup: biases, ones, ACT table warm ----
            qb_sb = setup.tile([P, 8], F32, tag="qb")
            nc.sync.dma_start(qb_sb[:], qb_d.ap().rearrange("(o p) -> p o", p=P))
            kb_sb = setup.tile([P, 8], F32, tag="kb")
            nc.sync.dma_start(kb_sb[:], kb_d.ap().rearrange("(o p) -> p o", p=P))
            vbB = setup.tile([P, D], F32, tag="vbB")
            obB = setup.tile([P, D], F32, tag="obB")
            vb_row = setup.tile([1, D], F32, tag="vbrow")
            nc.sync.dma_start(vb_row[:], vb_d.ap()[None, :])
            nc.gpsimd.partition_broadcast(vbB[:], vb_row[:])
            ob_row = setup.tile([1, D], F32, tag="obrow")
            nc.sync.dma_start(ob_row[:], ob_d.ap()[None, :])
            nc.gpsimd.partition_broadcast(obB[:], ob_row[:])

            onesF = setup.tile([P, P], F32, tag="onesF")
            nc.gpsimd.memset(onesF[:], 1.0)
            warm = setup.tile([1, 8], BF16, tag="warm")
            nc.scalar.activation(warm[:], onesF[0:1, 0:8], Exp)

            # ones columns of v65 (col 64 of each head block)
            nc.vector.tensor_copy(
                v65[:].rearrange("p t (h x) -> p t h x", x=65)[:, :, :, 64:65],
                onesF[:].rearrange("p (t h x) -> p t h x", t=8, h=16))

            # ---- V projection (with token-gated weight loads) ----
            v65v = v65[:].rearrange("p t (h x) -> p t h x", x=65)

            def gated_dma(dst_tile, dst_ap, src_ap):
                nc.vector.tensor_copy(dst_tile[0:1, 0, 0:1],
                                      v65[0:1, 0, 0:1])
                nc.sync.dma_start(dst_ap, src_ap)

            for half in range(2):
                for ti in range(8):
                    pp = psmm.tile([P, 512], F32, tag="pp",
                                   name=f"ppv{half}_{ti}")
                    for di in range(8):
                        nc.tensor.matmul(
                            pp[:],
                            kvT[:, di, ti * P:(ti + 1) * P],
                            wv[:, di, half * 512:(half + 1) * 512],
                            start=(di == 0), stop=(di == 7),
                        )
                    nc.vector.tensor_tensor(
                        v65v[:, ti, half * 8:(half + 1) * 8, 0:64], pp[:],
                        vbB[:, half * 512:(half + 1) * 512], add)
                    if half == 0 and ti == 0:
                        gated_dma(wk, wk[:],
                                  wk_t.ap().rearrange("(dd p) o -> p dd o", p=P))
                    elif half == 0 and ti == 2:
                        gated_dma(hidT, hidT[:],
                                  hid_t.ap().rearrange("(dd p) t -> p dd t", p=P))
                    elif half == 0 and ti == 5:
                        gated_dma(wq, wq[:],
                                  wq_t.ap().rearrange("(dd p) o -> p dd o", p=P))
                    elif half == 1 and ti == 1:
                        gated_dma(wo, wo[:],
                                  wo_t.ap().rearrange("(dd p) o -> p dd o", p=P))

            # ---- K/Q projections (pair 0 now, rest interleaved) ----
            def emit_kproj(hp, nk):
                pp = psmm.tile([P, 512], F32, tag="pp", name=f"ppk{hp}_{nk}")
                for di in range(8):
                    nc.tensor.matmul(
                        pp[:],
                        wk[:, di, hp * P:(hp + 1) * P],
                        kvT[:, di, nk * 512:(nk + 1) * 512],
                        start=(di == 0), stop=(di == 7),
                    )
                nc.vector.tensor_scalar(
                    KT[:, hp, nk * 512:(nk + 1) * 512], pp[:],
                    kb_sb[:, hp:hp + 1], None, add)

            def emit_qproj(hp):
                pq = psmm.tile([P, 512], F32, tag="pp", name=f"ppq{hp}")
                for di in range(8):
                    nc.tensor.matmul(
                        pq[:],
                        wq[:, di, hp * P:(hp + 1) * P],
                        hidT[:, di, :],
                        start=(di == 0), stop=(di == 7),
                    )
                nc.vector.tensor_scalar(qT[:, hp, :], pq[:],
                                        qb_sb[:, hp:hp + 1], None, add)

            emit_kproj(0, 0)
            emit_kproj(0, 1)
            emit_qproj(0)

            # normalization: all off-PE. Copy the sums row out of PSUM, DMA it
            # to partition 0, gpsimd-broadcast the raw sums, then a 64-lane
            # approx reciprocal (18-bit accurate, plenty for well-conditioned
            # positive denominators) and normalize-on-evict.
            def emit_norm(hp, ctx_ps):
                for hh in range(2):
                    cp = ctx_ps[hh]
                    srow = normp.tile([65, 512], F32, tag="srow",
                                      name=f"srow{hp}_{hh}")
                    nc.vector.tensor_copy(srow[64:65, :], cp[64:65, :])
                    r0 = normp.tile([1, 512], F32, tag="r0",
                                    name=f"r0_{hp}_{hh}")
                    nc.sync.dma_start(r0[:], srow[64:65, :])
                    bc = normp.tile([64, 512], F32, tag="bc",
                                    name=f"bc{hp}_{hh}")
                    nc.gpsimd.partition_broadcast(bc[:], r0[:])
                    rc = normp.tile([64, 512], F32, tag="rc",
                                    name=f"rc{hp}_{hh}")
                    nc.vector.reciprocal_approx_fast(rc[:], bc[:])
                    if hh == 0:
                        nc.vector.tensor_tensor(
                            ctxT[0:64, hp, :], cp[0:64, :], rc[:], mult)
                    else:
                        stg = normp.tile([64, 512], BF16, tag="stg",
                                         name=f"stg{hp}")
                        nc.vector.tensor_tensor(stg[:], cp[0:64, :], rc[:], mult)
                        nc.sync.dma_start(ctxT[64:128, hp, :], stg[:])

            # out-projection chunk helpers (epilogue + hp7 partials)
            def o_chunk_matmuls(po, half, mi, fjs, start0, stop7):
                for fj in fjs:
                    nc.tensor.matmul(
                        po[:],
                        ctxT[:, fj, mi * P:(mi + 1) * P],
                        wo[:, fj, half * 512:(half + 1) * 512],
                        start=(fj == 0 and start0), stop=(fj == 7 and stop7),
                    )

            def o_chunk_finish(po, half, mi):
                ot = outp.tile([P, 512], F32, tag="ot")
                nc.vector.tensor_tensor(
                    ot[:], po[:], obB[:, half * 512:(half + 1) * 512], add)
                nc.sync.dma_start(
                    out_s.ap().rearrange("(mm p) d -> p mm d", p=P)[
                        :, mi, half * 512:(half + 1) * 512],
                    ot[:])

            # ---- main attention loop (ctx pipelined 1 iter behind) ----
            ctx_tiles = {}
            pend = None  # (hp, t, [at_e, at_o])
            opart = []   # hp7 partial out-proj chunks: (po, half, mi)

            def emit_ctx(hp, t, ats):
                for hh in range(2):
                    h = 2 * hp + hh
                    nc.tensor.matmul(
                        ctx_tiles[hp][hh][:],
                        v65[:, t, h * 65:(h + 1) * 65],
                        ats[hh][:],
                        start=(t == 0), stop=(t == 7),
                    )

            for hp in range(8):
                nxt = hp + 1
                ctx_tiles[hp] = [psctx.tile([65, 512], F32, tag="ctx",
                                            name=f"ctx{hp}_{i}")
                                 for i in range(2)]
                for t in range(8):
                    ats = []
                    for hh in range(2):
                        lo = 64 * hh
                        sc = pssc.tile([P, 512], F32, tag="sc",
                                       name=f"sc{hp}_{t}_{hh}")
                        nc.tensor.matmul(
                            sc[:],
                            KT[lo:lo + 64, hp, t * P:(t + 1) * P],
                            qT[lo:lo + 64, hp, :],
                            start=True, stop=True,
                        )
                        at = attnp.tile([P, 512], BF16, tag="at")
                        nc.scalar.activation(at[:], sc[:], Exp)
                        ats.append(at)
                    if pend is not None:
                        phh, pt, pats = pend
                        emit_ctx(phh, pt, pats)
                        if pt == 7:
                            emit_norm(phh, ctx_tiles[phh])
                    if nxt < 8:
                        if t == 2:
                            emit_kproj(nxt, 0)
                        elif t == 4:
                            emit_kproj(nxt, 1)
                        elif t == 6:
                            emit_qproj(nxt)
                    else:
                        # keep PE fed while ACT drains: pre-accumulate the
                        # first two out-proj chunks over fj=0..6
                        if t == 2 or t == 4:
                            mi = 0 if t == 2 else 1
                            po = psmm.tile([P, 512], F32, tag="pp",
                                           name=f"ppo0_{mi}")
                            o_chunk_matmuls(po, 0, mi, range(7), True, False)
                            opart.append((po, 0, mi))
                    pend = (hp, t, ats)

            emit_ctx(7, 7, pend[2])
            emit_norm(7, ctx_tiles[7])

            # ---- epilogue: finish out projection ----
            for po, half, mi in opart:
                o_chunk_matmuls(po, half, mi, [7], False, True)
                o_chunk_finish(po, half, mi)
            for half in range(2):
                for mi in range(4):
                    if half == 0 and mi < 2:
                        continue
                    po = psmm.tile([P, 512], F32, tag="pp",
                                   name=f"ppo{half}_{mi}")
                    o_chunk_matmuls(po, half, mi, range(8), True, True)
                    o_chunk_finish(po, half, mi)

    nc.compile()
    return nc


def _prep_inputs(hidden_states, key_value_states, q_weight, q_bias,
                 kv_weight, kv_bias, out_weight, out_bias):
    f32 = np.float32
    bf16 = ml_dtypes.bfloat16
    hid = np.asarray(hidden_states, f32).reshape(B * LQ, D)
    kv = np.asarray(key_value_states, f32).reshape(B * LK, D)
    scale = f32(1.0 / 8.0)

    # de-interleave kv rows: row e <-> (h=e//128, j=(e%128)//64, d=e%64)
    e = np.arange(2 * D)
    kmask = (e % 128) < 64
    kidx, vidx = e[kmask], e[~kmask]
    kvw = np.asarray(kv_weight, f32)
    kvb = np.asarray(kv_bias, f32)

    shared = {
        "wq_t": np.ascontiguousarray((np.asarray(q_weight, f32) * scale).T.astype(bf16)),
        "wk_t": np.ascontiguousarray(kvw[kidx].T.astype(bf16)),
        "wv_t": np.ascontiguousarray(kvw[vidx].T.astype(bf16)),
        "wo_t": np.ascontiguousarray(np.asarray(out_weight, f32).T.astype(bf16)),
        "qb": np.ascontiguousarray(np.asarray(q_bias, f32) * scale),
        "kb": np.ascontiguousarray(kvb[kidx]),
        "vb": np.ascontiguousarray(kvb[vidx]),
        "ob": np.ascontiguousarray(np.asarray(out_bias, f32)),
    }
    kvT_by_batch = [
        np.ascontiguousarray(kv[b * LK:(b + 1) * LK].T.astype(bf16))
        for b in range(B)
    ]
    in_maps = []
    for c in range(NCORES):
        b = c // 2
        m = dict(shared)
        m["hid_t"] = np.ascontiguousarray(
            hid[c * TQ:(c + 1) * TQ].T.astype(bf16))
        m["kv_t"] = kvT_by_batch[b]
        in_maps.append(m)
    return in_maps


def kernel(hidden_states, key_value_states, q_weight, q_bias,
           kv_weight, kv_bias, out_weight, out_bias, _trace=False):
    if "nc" not in _CACHE:
        _CACHE["nc"] = _build_core_program()
    nc = _CACHE["nc"]
    in_maps = _prep_inputs(hidden_states, key_value_states, q_weight, q_bias,
                           kv_weight, kv_bias, out_weight, out_bias)
    res = bass_utils.run_bass_kernel_spmd(
        nc, in_maps, core_ids=list(range(NCORES)), trace=_trace)
    _CACHE["last_result"] = res
    out = np.concatenate([r["out_s"] for r in res.results], axis=0)
    return out.reshape(B, LQ, D)
